# revision 5
# baseline (speedup 1.0000x reference)
"""Deformable conv block (offset conv -> bilinear sampling -> 3x3 deform conv
-> BatchNorm + ReLU) on 8 Trainium2 NeuronCores.

Sharding: data-parallel over (image-pair, row-quarter). Core c handles images
(2*(c//4), 2*(c//4)+1) stacked on the partition dim (2 x 64 channels = 128
partitions), output rows [32*(c%4), 32*(c%4)+32).

Algorithm (bf16, exact piecewise-linear bilinear for |offset| < 2):
  off = conv3x3(x, offset_w) + b                     (PE, bf16)
  per tap t with offsets (dy, dx), define relu fields
    BP  = clamp(d, 0, 1)      BM' = clamp(d, -1, 0)   (= -clamp(-d,0,1))
    A1P = relu(d - 1)         A1M' = min(d + 1, 0)    (= -relu(-d-1))
  row lerp (exact for |dy|<2), per col shift j in {-2..2}:
    vc_j = BYP*Dy(r0)_j + BYM'*Dy(r0-1)_j  [+ A1YP*Dy(r0+1)_j + A1YM'*Dy(r0-2)_j
                                             for j in {-1,0,1}]
  col combine (exact given no position has both |dy|>1 and |dx|>1):
    W_j = x_j + vc_j
    S = W0 + BXP*(W1-W0) + BXM'*(W0-Wm1) + A1XP*(W2-W1) + A1XM'*(Wm1-Wm2)
  einsum: acc += W_t.T @ S_t   (PE, PSUM accumulation, bf16)
  BN stats via activation accum_out, 8-core AllReduce, affine+ReLU as one
  scalar-engine pass.
Dy/Dx are first-difference tensors of the padded x; *_o copies shifted by one
column keep every bf16 window 4B-aligned (DVE 2x perf mode).
"""
import os
import numpy as np

C, K2, H, W, B = 64, 9, 128, 128, 4
NCORES = 8
RPC = 32          # output rows per core
QR = 8            # rows per quarter-chunk
NQ = RPC // QR    # 4 quarters
PITCH = 134       # padded col pitch; col index = 3 + w
XROWS = RPC + 6   # 3-row halo each side
EPS = 1e-5
NPOS = float(B * H * W)

_CACHE = {}


def _build_program():
    from contextlib import ExitStack
    import concourse.bass as bass
    import concourse.tile as tile
    from concourse import bacc, mybir

    f32 = mybir.dt.float32
    bf16 = mybir.dt.bfloat16
    AF = mybir.ActivationFunctionType
    OP = mybir.AluOpType

    N_CORR_G = int(os.environ.get("N_CORR_G", "3"))
    N_PROD_G = int(os.environ.get("N_PROD_G", "0"))

    nc = bacc.Bacc(
        "TRN2",
        target_bir_lowering=False,
        debug=False,
        enable_asserts=False,
        num_devices=NCORES,
    )

    xs_d = nc.dram_tensor("xs", (128, XROWS * PITCH), bf16, kind="ExternalInput")
    ow_d = nc.dram_tensor("ow", (128, K2 * 18), bf16, kind="ExternalInput")
    sel_d = nc.dram_tensor("sel", (50, 18 * 128), bf16, kind="ExternalInput")
    wt_d = nc.dram_tensor("wt", (128, K2 * 64), bf16, kind="ExternalInput")
    ob_d = nc.dram_tensor("ob", (50, 1), f32, kind="ExternalInput")
    gb_d = nc.dram_tensor("gb", (64, 2), f32, kind="ExternalInput")
    out_d = nc.dram_tensor("out", (128, RPC * 128), bf16, kind="ExternalOutput")
    stats_in_d = nc.dram_tensor("stats_in", (128, 2), f32, kind="Internal")
    stats_sh_d = nc.dram_tensor(
        "stats_sh", (128, 2), f32, kind="Internal", addr_space="Shared"
    )

    with tile.TileContext(nc) as tc, ExitStack() as ctx:
        consts = ctx.enter_context(tc.tile_pool(name="consts", bufs=1))
        offc_pool = ctx.enter_context(tc.tile_pool(name="offc", bufs=2))
        offr_pool = ctx.enter_context(tc.tile_pool(name="offr", bufs=2))
        f_pool = ctx.enter_context(tc.tile_pool(name="fld", bufs=2))
        vc_pool = ctx.enter_context(tc.tile_pool(name="vc", bufs=1))
        pr_pool = ctx.enter_context(tc.tile_pool(name="pr", bufs=1))
        cr_pool = ctx.enter_context(tc.tile_pool(name="cr", bufs=2))
        s_pool = ctx.enter_context(tc.tile_pool(name="s", bufs=2))
        stat_pool = ctx.enter_context(tc.tile_pool(name="stat", bufs=1))
        ps_off = ctx.enter_context(tc.tile_pool(name="ps_off", bufs=1, space="PSUM"))
        ps_sel = ctx.enter_context(tc.tile_pool(name="ps_sel", bufs=1, space="PSUM"))
        ps_acc = ctx.enter_context(tc.tile_pool(name="ps_acc", bufs=1, space="PSUM"))

        xs = consts.tile([128, XROWS, PITCH], bf16, tag="xs")
        ow = consts.tile([128, K2 * 18], bf16, tag="ow")
        sel = consts.tile([50, 18 * 128], bf16, tag="sel")
        wt = consts.tile([128, K2 * 64], bf16, tag="wt")
        ob = consts.tile([50, 1], f32, tag="ob")
        gb = consts.tile([64, 2], f32, tag="gb")
        nc.sync.dma_start(xs[:].rearrange("p r c -> p (r c)"), xs_d.ap())
        nc.sync.dma_start(ow[:], ow_d.ap())
        nc.sync.dma_start(sel[:], sel_d.ap())
        nc.sync.dma_start(wt[:], wt_d.ap())
        nc.sync.dma_start(ob[:], ob_d.ap())
        nc.sync.dma_start(gb[:], gb_d.ap())

        # difference tensors + odd-phase copies (alignment)
        dy_e = consts.tile([128, XROWS - 1, PITCH], bf16, tag="dy_e")
        dy_o = consts.tile([128, XROWS - 1, PITCH - 1], bf16, tag="dy_o")
        dx_e = consts.tile([128, XROWS, PITCH - 1], bf16, tag="dx_e")
        dx_o = consts.tile([128, XROWS, PITCH - 2], bf16, tag="dx_o")
        xs_o = consts.tile([128, XROWS, PITCH - 1], bf16, tag="xs_o")
        nc.vector.tensor_tensor(
            dy_e[:], xs[:, 1:XROWS, :], xs[:, 0 : XROWS - 1, :], OP.subtract
        )
        nc.vector.tensor_tensor(
            dx_e[:], xs[:, :, 1:PITCH], xs[:, :, 0 : PITCH - 1], OP.subtract
        )
        nc.sync.dma_start(dy_o[:], dy_e[:, :, 1:])
        nc.sync.dma_start(dx_o[:], dx_e[:, :, 1:])
        nc.sync.dma_start(xs_o[:], xs[:, :, 1:])

        def dyw(r, c):  # Dy window [128, QR, 128] at row base r, col base c
            if c % 2 == 0:
                return dy_e[:, r : r + QR, c : c + 128]
            return dy_o[:, r : r + QR, c - 1 : c - 1 + 128]

        def dxw(r, c):
            if c % 2 == 0:
                return dx_e[:, r : r + QR, c : c + 128]
            return dx_o[:, r : r + QR, c - 1 : c - 1 + 128]

        def xsw(r, c, rows=QR):
            if c % 2 == 0:
                return xs[:, r : r + rows, c : c + 128]
            return xs_o[:, r : r + rows, c - 1 : c - 1 + 128]

        out_pre = consts.tile([128, RPC * 128], bf16, tag="out_pre")
        scr = consts.tile([128, 512], bf16, tag="scr")
        sums = stat_pool.tile([128, 2, 8], f32, tag="sums")

        for q in range(NQ):
            # ---- offset conv: offp [50, (h2,4,128)] ----
            offp = ps_off.tile([50, 1024], f32, tag="offp", name="offp")
            for img in range(2):
                pb = img * 64
                obb = img * 32
                for t9 in range(K2):
                    ti, tj = t9 // 3, t9 % 3
                    for h2 in range(2):
                        rhs = xsw(8 * q + 2 + ti + 4 * h2, 2 + tj, rows=4)[
                            pb : pb + 64
                        ]
                        nc.tensor.matmul(
                            offp[obb : obb + 18, h2 * 512 : (h2 + 1) * 512],
                            ow[pb : pb + 64, t9 * 18 : (t9 + 1) * 18],
                            rhs,
                            start=(t9 == 0),
                            stop=(t9 == K2 - 1),
                        )
            offc = offc_pool.tile([50, 1024], bf16, tag="offc")
            nc.scalar.activation(offc[:], offp[:], AF.Identity, bias=ob[:], scale=1.0)

            acc = [
                ps_acc.tile([128, 512], f32, tag=f"acc{h2}", name=f"acc{h2}")
                for h2 in range(2)
            ]

            for t in range(K2):
                ki, kj = t // 3, t % 3
                rb = 8 * q + 2 + ki
                c0 = 2 + kj
                # ---- sel broadcast -> PSUM [128, (dyx, h2, 4, 128)] ----
                sps = ps_sel.tile([128, 2048], f32, tag="sps", name="sps")
                for dyx in range(2):
                    for h2 in range(2):
                        nc.tensor.matmul(
                            sps[:, dyx * 1024 + h2 * 512 : dyx * 1024 + (h2 + 1) * 512],
                            sel[:, (2 * t + dyx) * 128 : (2 * t + dyx + 1) * 128],
                            offc[:, h2 * 512 : (h2 + 1) * 512],
                            start=True,
                            stop=True,
                        )
                offr = offr_pool.tile([128, 2, QR, 128], bf16, tag="offr")
                nc.scalar.copy(offr[:].rearrange("p a r c -> p (a r c)"), sps[:])

                # ---- fields F[s, dyx, QR, 128]: s = BM', BP, A1M', A1P ----
                F = f_pool.tile([128, 4, 2, QR, 128], bf16, tag="F")
                nc.vector.tensor_scalar(F[:, 1], offr[:], 1.0, 0.0, OP.min, OP.max)
                nc.vector.tensor_scalar(F[:, 0], offr[:], -1.0, 0.0, OP.max, OP.min)
                nc.gpsimd.tensor_scalar(F[:, 3], offr[:], 1.0, 0.0, OP.subtract, OP.max)
                nc.gpsimd.tensor_scalar(F[:, 2], offr[:], 1.0, 0.0, OP.add, OP.min)
                BYP, BYM = F[:, 1, 0], F[:, 0, 0]
                A1YP, A1YM = F[:, 3, 0], F[:, 2, 0]

                # ---- row stages ----
                tp = pr_pool.tile([128, 5, QR, 128], bf16, tag="tp", name="tp")
                tq = pr_pool.tile([128, 5, QR, 128], bf16, tag="tq", name="tq")
                tp2 = cr_pool.tile([128, 3, QR, 128], bf16, tag="tp2", name="tp2")
                tq2 = cr_pool.tile([128, 3, QR, 128], bf16, tag="tq2", name="tq2")
                vcs = vc_pool.tile([128, 5, QR, 128], bf16, tag="vcs", name="vcs")
                ng = 0
                for s in range(3):  # corr products, j = s-1
                    cj = c0 + s - 1
                    eng1 = nc.gpsimd if ng < N_CORR_G else nc.vector
                    eng2 = nc.gpsimd if ng + 1 < N_CORR_G else nc.vector
                    ng += 2
                    eng1.tensor_tensor(tp2[:, s], A1YP, dyw(rb + 1, cj), OP.mult)
                    eng2.tensor_tensor(tq2[:, s], A1YM, dyw(rb - 2, cj), OP.mult)
                np_ = 0
                for s in range(5):  # core products, j = s-2
                    cj = c0 + s - 2
                    eng1 = nc.gpsimd if np_ < N_PROD_G else nc.vector
                    eng2 = nc.gpsimd if np_ + 1 < N_PROD_G else nc.vector
                    np_ += 2
                    eng1.tensor_tensor(tp[:, s], BYP, dyw(rb, cj), OP.mult)
                    eng2.tensor_tensor(tq[:, s], BYM, dyw(rb - 1, cj), OP.mult)
                nc.vector.tensor_tensor(vcs[:], tp[:], tq[:], OP.add)
                nc.vector.tensor_tensor(vcs[:, 1:4], vcs[:, 1:4], tp2[:], OP.add)
                nc.vector.tensor_tensor(vcs[:, 1:4], vcs[:, 1:4], tq2[:], OP.add)

                # ---- col stage ----
                tpair = s_pool.tile([128, 2, QR, 128], bf16, tag="tpair", name="tpair")
                upair = s_pool.tile([128, 2, QR, 128], bf16, tag="upair", name="upair")
                nc.vector.tensor_tensor(tpair[:], vcs[:, 2:4], vcs[:, 1:3], OP.subtract)
                nc.vector.tensor_tensor(tpair[:, 0], tpair[:, 0], dxw(rb, c0 - 1), OP.add)
                nc.vector.tensor_tensor(tpair[:, 1], tpair[:, 1], dxw(rb, c0), OP.add)
                nc.vector.tensor_tensor(upair[:], vcs[:, 1:5:3], vcs[:, 0:4:3], OP.subtract)
                nc.vector.tensor_tensor(upair[:, 0], upair[:, 0], dxw(rb, c0 - 2), OP.add)
                nc.vector.tensor_tensor(upair[:, 1], upair[:, 1], dxw(rb, c0 + 1), OP.add)
                Sp = s_pool.tile([128, 2, QR, 128], bf16, tag="Sp", name="Sp")
                nc.vector.tensor_tensor(Sp[:], F[:, 0:2, 1], tpair[:], OP.mult)
                nc.vector.tensor_tensor(upair[:], F[:, 2:4, 1], upair[:], OP.mult)
                nc.vector.tensor_tensor(Sp[:], Sp[:], upair[:], OP.add)
                S = s_pool.tile([128, QR, 128], bf16, tag="S", name="S")
                nc.vector.tensor_tensor(S[:], Sp[:, 0], Sp[:, 1], OP.add)
                nc.vector.tensor_tensor(S[:], S[:], vcs[:, 2], OP.add)
                nc.vector.tensor_tensor(S[:], S[:], xsw(rb, c0), OP.add)

                # ---- einsum into PSUM ----
                Sf = S[:].rearrange("p r c -> p (r c)")
                for img in range(2):
                    pb = img * 64
                    for h2 in range(2):
                        nc.tensor.matmul(
                            acc[h2][pb : pb + 64, :],
                            wt[pb : pb + 64, t * 64 : (t + 1) * 64],
                            Sf[pb : pb + 64, h2 * 512 : (h2 + 1) * 512],
                            start=(t == 0),
                            stop=(t == K2 - 1),
                        )

            # ---- PSUM -> out_pre with BN partial sums for free ----
            for h2 in range(2):
                sl = 2 * q + h2
                nc.scalar.activation(
                    out_pre[:, q * 1024 + h2 * 512 : q * 1024 + (h2 + 1) * 512],
                    acc[h2][:],
                    AF.Copy,
                    accum_out=sums[:, 0, sl : sl + 1],
                )
                nc.scalar.activation(
                    scr[:], acc[h2][:], AF.Square, accum_out=sums[:, 1, sl : sl + 1]
                )

        # ---- BatchNorm ----
        st2 = stat_pool.tile([128, 2], f32, tag="st2")
        nc.vector.tensor_reduce(
            st2[:, 0:1], sums[:, 0], mybir.AxisListType.X, OP.add
        )
        nc.vector.tensor_reduce(
            st2[:, 1:2], sums[:, 1], mybir.AxisListType.X, OP.add
        )
        nc.sync.dma_start(stats_in_d.ap(), st2[:])
        nc.gpsimd.collective_compute(
            "AllReduce", OP.add, [list(range(NCORES))],
            ins=[stats_in_d.ap()], outs=[stats_sh_d.ap()],
        )
        tot_a = stat_pool.tile([64, 2], f32, tag="tot_a")
        tot_b = stat_pool.tile([64, 2], f32, tag="tot_b")
        nc.sync.dma_start(tot_a[:], stats_sh_d.ap()[0:64, :])
        nc.sync.dma_start(tot_b[:], stats_sh_d.ap()[64:128, :])
        tot64 = stat_pool.tile([64, 2], f32, tag="tot64")
        nc.vector.tensor_tensor(tot64[:], tot_a[:], tot_b[:], OP.add)
        fin = stat_pool.tile([64, 8], f32, tag="fin")
        mu = fin[:, 0:1]; ex2 = fin[:, 1:2]; m2 = fin[:, 2:3]; var = fin[:, 3:4]
        inv = fin[:, 4:5]; rstd = fin[:, 5:6]; sc = fin[:, 6:7]; tc_ = fin[:, 7:8]
        nc.vector.tensor_scalar_mul(mu, tot64[:, 0:1], 1.0 / NPOS)
        nc.vector.tensor_scalar_mul(ex2, tot64[:, 1:2], 1.0 / NPOS)
        nc.vector.tensor_tensor(m2, mu, mu, OP.mult)
        nc.vector.tensor_tensor(var, ex2, m2, OP.subtract)
        nc.vector.tensor_scalar_add(var, var, EPS)
        nc.vector.reciprocal(inv, var)
        nc.scalar.activation(rstd, inv, AF.Sqrt)
        nc.vector.tensor_tensor(sc, rstd, gb[:, 0:1], OP.mult)
        nc.vector.tensor_tensor(tc_, mu, sc, OP.mult)
        nc.vector.tensor_tensor(tc_, gb[:, 1:2], tc_, OP.subtract)
        st = stat_pool.tile([128, 2], f32, tag="st")
        nc.sync.dma_start(st[0:64, :], fin[:, 6:8])
        nc.sync.dma_start(st[64:128, :], fin[:, 6:8])
        nc.scalar.activation(
            out_pre[:], out_pre[:], AF.Relu, bias=st[:, 1:2], scale=st[:, 0:1]
        )
        nc.sync.dma_start(out_d.ap(), out_pre[:])

    nc.compile()
    return nc


def _shard_inputs(x, offset_w, offset_b, dcn_w, gamma, beta):
    """Build the 8 per-core input maps."""
    import ml_dtypes

    bf = ml_dtypes.bfloat16
    x = np.asarray(x, np.float32)
    ow_full = np.asarray(offset_w, np.float32)
    ob_full = np.asarray(offset_b, np.float32)
    wt_full = np.asarray(dcn_w, np.float32)

    ow1 = ow_full.transpose(1, 2, 3, 0).reshape(64, K2 * 18)
    ow = np.concatenate([ow1, ow1], axis=0).astype(bf)
    wt1 = wt_full.transpose(1, 2, 3, 0).reshape(64, K2 * 64)
    wt = np.concatenate([wt1, wt1], axis=0).astype(bf)
    ob = np.zeros((50, 1), np.float32)
    ob[0:18, 0] = ob_full
    ob[32:50, 0] = ob_full
    sel = np.zeros((50, 18 * 128), np.float32)
    for t in range(K2):
        for dyx in range(2):
            j = (2 * t + dyx) * 128
            sel[2 * t + dyx, j : j + 64] = 1.0
            sel[32 + 2 * t + dyx, j + 64 : j + 128] = 1.0
    sel = sel.astype(bf)
    gb = np.stack(
        [np.asarray(gamma, np.float32), np.asarray(beta, np.float32)], axis=1
    ).copy()

    in_maps = []
    for core in range(NCORES):
        pair, q = core // 4, core % 4
        shard = np.zeros((128, XROWS, PITCH), np.float32)
        r_lo = 32 * q - 3
        for blk in range(2):
            img = 2 * pair + blk
            g0, g1 = max(0, r_lo), min(H, r_lo + XROWS)
            shard[blk * 64 : (blk + 1) * 64, g0 - r_lo : g1 - r_lo, 3:131] = x[
                img, :, g0:g1, :
            ]
        in_maps.append(
            dict(
                xs=shard.reshape(128, XROWS * PITCH).astype(bf),
                ow=ow, sel=sel, wt=wt, ob=ob, gb=gb,
            )
        )
    return in_maps


def kernel(x, offset_w, offset_b, dcn_w, gamma, beta):
    from concourse.bass_utils import run_bass_kernel_spmd

    if "nc" not in _CACHE:
        _CACHE["nc"] = _build_program()
    nc = _CACHE["nc"]

    in_maps = _shard_inputs(x, offset_w, offset_b, dcn_w, gamma, beta)
    res = run_bass_kernel_spmd(nc, in_maps, core_ids=list(range(NCORES)))
    out = np.zeros((B, C, H, W), np.float32)
    for core in range(NCORES):
        pair, q = core // 4, core % 4
        o = res.results[core]["out"].astype(np.float32).reshape(128, RPC, 128)
        for blk in range(2):
            out[2 * pair + blk, :, 32 * q : 32 * q + 32, :] = o[
                blk * 64 : (blk + 1) * 64
            ]
    return out


# revision 8
# speedup vs baseline: 2.2522x; 2.2522x over previous
"""Deformable conv block (offset conv -> bilinear sampling -> 3x3 deform conv
-> BatchNorm + ReLU) on 8 Trainium2 NeuronCores.

Sharding: data-parallel over (image-pair, row-quarter). Core c handles images
(2*(c//4), 2*(c//4)+1) stacked on the partition dim (2 x 64 channels = 128
partitions), output rows [32*(c%4), 32*(c%4)+32).

Algorithm (bf16, exact piecewise-linear bilinear for |offset| < 2):
  off = conv3x3(x, offset_w) + b                     (PE, bf16)
  per tap t with offsets (dy, dx), define relu fields
    BP  = clamp(d, 0, 1)      BM' = clamp(d, -1, 0)   (= -clamp(-d,0,1))
    A1P = relu(d - 1)         A1M' = min(d + 1, 0)    (= -relu(-d-1))
  row lerp (exact for |dy|<2), per col shift j in {-2..2}:
    vc_j = BYP*Dy(r0)_j + BYM'*Dy(r0-1)_j  [+ A1YP*Dy(r0+1)_j + A1YM'*Dy(r0-2)_j
                                             for j in {-1,0,1}]
  col combine (exact given no position has both |dy|>1 and |dx|>1):
    W_j = x_j + vc_j
    S = W0 + BXP*(W1-W0) + BXM'*(W0-Wm1) + A1XP*(W2-W1) + A1XM'*(Wm1-Wm2)
  einsum: acc += W_t.T @ S_t   (PE, PSUM accumulation, bf16)
  BN stats via activation accum_out, 8-core AllReduce, affine+ReLU as one
  scalar-engine pass.
Dy/Dx are first-difference tensors of the padded x; *_o copies shifted by one
column keep every bf16 window 4B-aligned (DVE 2x perf mode).
"""
import os
import numpy as np

C, K2, H, W, B = 64, 9, 128, 128, 4
NCORES = 8
RPC = 32          # output rows per core
QR = 8            # rows per quarter-chunk
NQ = RPC // QR    # 4 quarters
PITCH = 134       # padded col pitch; col index = 3 + w
XROWS = RPC + 6   # 3-row halo each side
EPS = 1e-5
NPOS = float(B * H * W)

_CACHE = {}


def _build_program():
    from contextlib import ExitStack
    import concourse.bass as bass
    import concourse.tile as tile
    from concourse import bacc, mybir

    f32 = mybir.dt.float32
    bf16 = mybir.dt.bfloat16
    AF = mybir.ActivationFunctionType
    OP = mybir.AluOpType

    N_CORR_G = int(os.environ.get("N_CORR_G", "6"))
    N_PROD_G = int(os.environ.get("N_PROD_G", "2"))

    nc = bacc.Bacc(
        "TRN2",
        target_bir_lowering=False,
        debug=False,
        enable_asserts=False,
        num_devices=NCORES,
    )

    xs_d = nc.dram_tensor("xs", (128, XROWS * PITCH), bf16, kind="ExternalInput")
    ow_d = nc.dram_tensor("ow", (128, K2 * 18), bf16, kind="ExternalInput")
    sel_d = nc.dram_tensor("sel", (50, 18 * 128), bf16, kind="ExternalInput")
    wt_d = nc.dram_tensor("wt", (128, K2 * 64), bf16, kind="ExternalInput")
    ob_d = nc.dram_tensor("ob", (50, 1), f32, kind="ExternalInput")
    gb_d = nc.dram_tensor("gb", (64, 2), f32, kind="ExternalInput")
    out_d = nc.dram_tensor("out", (128, RPC * 128), bf16, kind="ExternalOutput")
    stats_in_d = nc.dram_tensor("stats_in", (128, 2), f32, kind="Internal")
    stats_sh_d = nc.dram_tensor(
        "stats_sh", (128, 2), f32, kind="Internal", addr_space="Shared"
    )

    with tile.TileContext(nc) as tc, ExitStack() as ctx:
        consts = ctx.enter_context(tc.tile_pool(name="consts", bufs=1))
        offc_pool = ctx.enter_context(tc.tile_pool(name="offc", bufs=2))
        offr_pool = ctx.enter_context(tc.tile_pool(name="offr", bufs=2))
        f_pool = ctx.enter_context(tc.tile_pool(name="fld", bufs=2))
        vc_pool = ctx.enter_context(tc.tile_pool(name="vc", bufs=1))
        pr_pool = ctx.enter_context(tc.tile_pool(name="pr", bufs=1))
        cr_pool = ctx.enter_context(tc.tile_pool(name="cr", bufs=2))
        s_pool = ctx.enter_context(tc.tile_pool(name="s", bufs=2))
        stat_pool = ctx.enter_context(tc.tile_pool(name="stat", bufs=1))
        ps_off = ctx.enter_context(tc.tile_pool(name="ps_off", bufs=1, space="PSUM"))
        ps_sel = ctx.enter_context(tc.tile_pool(name="ps_sel", bufs=1, space="PSUM"))
        ps_acc = ctx.enter_context(tc.tile_pool(name="ps_acc", bufs=1, space="PSUM"))

        xs = consts.tile([128, XROWS, PITCH], bf16, tag="xs")
        ow = consts.tile([128, K2 * 18], bf16, tag="ow")
        sel = consts.tile([50, 18 * 128], bf16, tag="sel")
        wt = consts.tile([128, K2 * 64], bf16, tag="wt")
        ob = consts.tile([50, 1], f32, tag="ob")
        gb = consts.tile([64, 2], f32, tag="gb")
        nc.sync.dma_start(xs[:].rearrange("p r c -> p (r c)"), xs_d.ap())
        nc.sync.dma_start(ow[:], ow_d.ap())
        nc.sync.dma_start(sel[:], sel_d.ap())
        nc.sync.dma_start(wt[:], wt_d.ap())
        nc.sync.dma_start(ob[:], ob_d.ap())
        nc.sync.dma_start(gb[:], gb_d.ap())

        # difference tensors + odd-phase copies; all tiles keep the even
        # PITCH=134 row pitch so every window row start stays 4B-aligned
        # (268B rows) and the DVE 2x bf16 perf mode engages.
        dy_e = consts.tile([128, XROWS - 1, PITCH], bf16, tag="dy_e")
        dy_o = consts.tile([128, XROWS - 1, PITCH], bf16, tag="dy_o")
        dx_e = consts.tile([128, XROWS, PITCH], bf16, tag="dx_e")
        dx_o = consts.tile([128, XROWS, PITCH], bf16, tag="dx_o")
        xs_o = consts.tile([128, XROWS, PITCH], bf16, tag="xs_o")
        nc.vector.tensor_tensor(
            dy_e[:], xs[:, 1:XROWS, :], xs[:, 0 : XROWS - 1, :], OP.subtract
        )
        nc.vector.tensor_tensor(
            dx_e[:, :, 0 : PITCH - 1],
            xs[:, :, 1:PITCH],
            xs[:, :, 0 : PITCH - 1],
            OP.subtract,
        )
        nc.sync.dma_start(dy_o[:, :, 0 : PITCH - 1], dy_e[:, :, 1:])
        nc.sync.dma_start(dx_o[:, :, 0 : PITCH - 2], dx_e[:, :, 1 : PITCH - 1])
        nc.sync.dma_start(xs_o[:, :, 0 : PITCH - 1], xs[:, :, 1:])

        def dyw(r, c):  # Dy window [128, QR, 128] at row base r, col base c
            if c % 2 == 0:
                return dy_e[:, r : r + QR, c : c + 128]
            return dy_o[:, r : r + QR, c - 1 : c - 1 + 128]

        def dxw(r, c):
            if c % 2 == 0:
                return dx_e[:, r : r + QR, c : c + 128]
            return dx_o[:, r : r + QR, c - 1 : c - 1 + 128]

        def xsw(r, c, rows=QR):
            if c % 2 == 0:
                return xs[:, r : r + rows, c : c + 128]
            return xs_o[:, r : r + rows, c - 1 : c - 1 + 128]

        out_pre = consts.tile([128, RPC * 128], bf16, tag="out_pre")
        scr = consts.tile([128, 512], bf16, tag="scr")
        sums = stat_pool.tile([128, 2, 8], f32, tag="sums")

        for q in range(NQ):
            # ---- offset conv: offp [50, (h2,4,128)] ----
            offp = ps_off.tile([50, 1024], f32, tag="offp", name="offp")
            for img in range(2):
                pb = img * 64
                obb = img * 32
                for t9 in range(K2):
                    ti, tj = t9 // 3, t9 % 3
                    for h2 in range(2):
                        rhs = xsw(8 * q + 2 + ti + 4 * h2, 2 + tj, rows=4)[
                            pb : pb + 64
                        ]
                        nc.tensor.matmul(
                            offp[obb : obb + 18, h2 * 512 : (h2 + 1) * 512],
                            ow[pb : pb + 64, t9 * 18 : (t9 + 1) * 18],
                            rhs,
                            start=(t9 == 0),
                            stop=(t9 == K2 - 1),
                        )
            offc = offc_pool.tile([50, 1024], bf16, tag="offc")
            nc.scalar.activation(offc[:], offp[:], AF.Identity, bias=ob[:], scale=1.0)

            acc = [
                ps_acc.tile([128, 512], f32, tag=f"acc{h2}", name=f"acc{h2}")
                for h2 in range(2)
            ]

            for t in range(K2):
                ki, kj = t // 3, t % 3
                rb = 8 * q + 2 + ki
                c0 = 2 + kj
                # ---- sel broadcast -> PSUM [128, (dyx, h2, 4, 128)] ----
                sps = ps_sel.tile([128, 2048], f32, tag="sps", name="sps")
                for dyx in range(2):
                    for h2 in range(2):
                        nc.tensor.matmul(
                            sps[:, dyx * 1024 + h2 * 512 : dyx * 1024 + (h2 + 1) * 512],
                            sel[:, (2 * t + dyx) * 128 : (2 * t + dyx + 1) * 128],
                            offc[:, h2 * 512 : (h2 + 1) * 512],
                            start=True,
                            stop=True,
                        )
                offr = offr_pool.tile([128, 2, QR, 128], bf16, tag="offr")
                nc.scalar.copy(offr[:].rearrange("p a r c -> p (a r c)"), sps[:])

                # ---- fields F[s, dyx, QR, 128]: s = BM', BP, A1M', A1P ----
                F = f_pool.tile([128, 4, 2, QR, 128], bf16, tag="F")
                nc.vector.tensor_scalar(F[:, 1], offr[:], 1.0, 0.0, OP.min, OP.max)
                nc.vector.tensor_scalar(F[:, 0], offr[:], -1.0, 0.0, OP.max, OP.min)
                # A1P = relu(d-1) = (d max 1) - 1 ;  A1M' = -relu(-d-1) = (d min -1) + 1
                nc.vector.tensor_scalar(F[:, 3], offr[:], 1.0, -1.0, OP.max, OP.add)
                nc.vector.tensor_scalar(F[:, 2], offr[:], -1.0, 1.0, OP.min, OP.add)
                BYP, BYM = F[:, 1, 0], F[:, 0, 0]
                A1YP, A1YM = F[:, 3, 0], F[:, 2, 0]

                # ---- row stages ----
                tp = pr_pool.tile([128, 5, QR, 128], bf16, tag="tp", name="tp")
                tq = pr_pool.tile([128, 5, QR, 128], bf16, tag="tq", name="tq")
                tp2 = cr_pool.tile([128, 3, QR, 128], bf16, tag="tp2", name="tp2")
                tq2 = cr_pool.tile([128, 3, QR, 128], bf16, tag="tq2", name="tq2")
                vcs = vc_pool.tile([128, 5, QR, 128], bf16, tag="vcs", name="vcs")
                ng = 0
                for s in range(3):  # corr products, j = s-1
                    cj = c0 + s - 1
                    eng1 = nc.gpsimd if ng < N_CORR_G else nc.vector
                    eng2 = nc.gpsimd if ng + 1 < N_CORR_G else nc.vector
                    ng += 2
                    eng1.tensor_tensor(tp2[:, s], A1YP, dyw(rb + 1, cj), OP.mult)
                    eng2.tensor_tensor(tq2[:, s], A1YM, dyw(rb - 2, cj), OP.mult)
                np_ = 0
                for s in range(5):  # core products, j = s-2
                    cj = c0 + s - 2
                    eng1 = nc.gpsimd if np_ < N_PROD_G else nc.vector
                    eng2 = nc.gpsimd if np_ + 1 < N_PROD_G else nc.vector
                    np_ += 2
                    eng1.tensor_tensor(tp[:, s], BYP, dyw(rb, cj), OP.mult)
                    eng2.tensor_tensor(tq[:, s], BYM, dyw(rb - 1, cj), OP.mult)
                nc.vector.tensor_tensor(vcs[:], tp[:], tq[:], OP.add)
                nc.vector.tensor_tensor(vcs[:, 1:4], vcs[:, 1:4], tp2[:], OP.add)
                nc.vector.tensor_tensor(vcs[:, 1:4], vcs[:, 1:4], tq2[:], OP.add)

                # ---- col stage ----
                tpair = s_pool.tile([128, 2, QR, 128], bf16, tag="tpair", name="tpair")
                upair = s_pool.tile([128, 2, QR, 128], bf16, tag="upair", name="upair")
                nc.vector.tensor_tensor(tpair[:], vcs[:, 2:4], vcs[:, 1:3], OP.subtract)
                nc.vector.tensor_tensor(tpair[:, 0], tpair[:, 0], dxw(rb, c0 - 1), OP.add)
                nc.vector.tensor_tensor(tpair[:, 1], tpair[:, 1], dxw(rb, c0), OP.add)
                nc.vector.tensor_tensor(upair[:], vcs[:, 1:5:3], vcs[:, 0:4:3], OP.subtract)
                nc.vector.tensor_tensor(upair[:, 0], upair[:, 0], dxw(rb, c0 - 2), OP.add)
                nc.vector.tensor_tensor(upair[:, 1], upair[:, 1], dxw(rb, c0 + 1), OP.add)
                Sp = s_pool.tile([128, 2, QR, 128], bf16, tag="Sp", name="Sp")
                nc.vector.tensor_tensor(Sp[:], F[:, 0:2, 1], tpair[:], OP.mult)
                nc.vector.tensor_tensor(upair[:], F[:, 2:4, 1], upair[:], OP.mult)
                nc.vector.tensor_tensor(Sp[:], Sp[:], upair[:], OP.add)
                S = s_pool.tile([128, QR, 128], bf16, tag="S", name="S")
                nc.vector.tensor_tensor(S[:], Sp[:, 0], Sp[:, 1], OP.add)
                nc.vector.tensor_tensor(S[:], S[:], vcs[:, 2], OP.add)
                nc.vector.tensor_tensor(S[:], S[:], xsw(rb, c0), OP.add)

                # ---- einsum into PSUM ----
                Sf = S[:].rearrange("p r c -> p (r c)")
                for img in range(2):
                    pb = img * 64
                    for h2 in range(2):
                        nc.tensor.matmul(
                            acc[h2][pb : pb + 64, :],
                            wt[pb : pb + 64, t * 64 : (t + 1) * 64],
                            Sf[pb : pb + 64, h2 * 512 : (h2 + 1) * 512],
                            start=(t == 0),
                            stop=(t == K2 - 1),
                        )

            # ---- PSUM -> out_pre with BN partial sums for free ----
            for h2 in range(2):
                sl = 2 * q + h2
                nc.scalar.activation(
                    out_pre[:, q * 1024 + h2 * 512 : q * 1024 + (h2 + 1) * 512],
                    acc[h2][:],
                    AF.Copy,
                    accum_out=sums[:, 0, sl : sl + 1],
                )
                nc.scalar.activation(
                    scr[:], acc[h2][:], AF.Square, accum_out=sums[:, 1, sl : sl + 1]
                )

        # ---- BatchNorm ----
        st2 = stat_pool.tile([128, 2], f32, tag="st2")
        nc.vector.tensor_reduce(
            st2[:, 0:1], sums[:, 0], mybir.AxisListType.X, OP.add
        )
        nc.vector.tensor_reduce(
            st2[:, 1:2], sums[:, 1], mybir.AxisListType.X, OP.add
        )
        nc.sync.dma_start(stats_in_d.ap(), st2[:])
        nc.gpsimd.collective_compute(
            "AllReduce", OP.add, [list(range(NCORES))],
            ins=[stats_in_d.ap()], outs=[stats_sh_d.ap()],
        )
        tot_a = stat_pool.tile([64, 2], f32, tag="tot_a")
        tot_b = stat_pool.tile([64, 2], f32, tag="tot_b")
        nc.sync.dma_start(tot_a[:], stats_sh_d.ap()[0:64, :])
        nc.sync.dma_start(tot_b[:], stats_sh_d.ap()[64:128, :])
        tot64 = stat_pool.tile([64, 2], f32, tag="tot64")
        nc.vector.tensor_tensor(tot64[:], tot_a[:], tot_b[:], OP.add)
        fin = stat_pool.tile([64, 8], f32, tag="fin")
        mu = fin[:, 0:1]; ex2 = fin[:, 1:2]; m2 = fin[:, 2:3]; var = fin[:, 3:4]
        inv = fin[:, 4:5]; rstd = fin[:, 5:6]; sc = fin[:, 6:7]; tc_ = fin[:, 7:8]
        nc.vector.tensor_scalar_mul(mu, tot64[:, 0:1], 1.0 / NPOS)
        nc.vector.tensor_scalar_mul(ex2, tot64[:, 1:2], 1.0 / NPOS)
        nc.vector.tensor_tensor(m2, mu, mu, OP.mult)
        nc.vector.tensor_tensor(var, ex2, m2, OP.subtract)
        nc.vector.tensor_scalar_add(var, var, EPS)
        nc.vector.reciprocal(inv, var)
        nc.scalar.activation(rstd, inv, AF.Sqrt)
        nc.vector.tensor_tensor(sc, rstd, gb[:, 0:1], OP.mult)
        nc.vector.tensor_tensor(tc_, mu, sc, OP.mult)
        nc.vector.tensor_tensor(tc_, gb[:, 1:2], tc_, OP.subtract)
        st = stat_pool.tile([128, 2], f32, tag="st")
        nc.sync.dma_start(st[0:64, :], fin[:, 6:8])
        nc.sync.dma_start(st[64:128, :], fin[:, 6:8])
        nc.scalar.activation(
            out_pre[:], out_pre[:], AF.Relu, bias=st[:, 1:2], scale=st[:, 0:1]
        )
        nc.sync.dma_start(out_d.ap(), out_pre[:])

    nc.compile()
    return nc


def _shard_inputs(x, offset_w, offset_b, dcn_w, gamma, beta):
    """Build the 8 per-core input maps."""
    import ml_dtypes

    bf = ml_dtypes.bfloat16
    x = np.asarray(x, np.float32)
    ow_full = np.asarray(offset_w, np.float32)
    ob_full = np.asarray(offset_b, np.float32)
    wt_full = np.asarray(dcn_w, np.float32)

    ow1 = ow_full.transpose(1, 2, 3, 0).reshape(64, K2 * 18)
    ow = np.concatenate([ow1, ow1], axis=0).astype(bf)
    wt1 = wt_full.transpose(1, 2, 3, 0).reshape(64, K2 * 64)
    wt = np.concatenate([wt1, wt1], axis=0).astype(bf)
    ob = np.zeros((50, 1), np.float32)
    ob[0:18, 0] = ob_full
    ob[32:50, 0] = ob_full
    sel = np.zeros((50, 18 * 128), np.float32)
    for t in range(K2):
        for dyx in range(2):
            j = (2 * t + dyx) * 128
            sel[2 * t + dyx, j : j + 64] = 1.0
            sel[32 + 2 * t + dyx, j + 64 : j + 128] = 1.0
    sel = sel.astype(bf)
    gb = np.stack(
        [np.asarray(gamma, np.float32), np.asarray(beta, np.float32)], axis=1
    ).copy()

    in_maps = []
    for core in range(NCORES):
        pair, q = core // 4, core % 4
        shard = np.zeros((128, XROWS, PITCH), np.float32)
        r_lo = 32 * q - 3
        for blk in range(2):
            img = 2 * pair + blk
            g0, g1 = max(0, r_lo), min(H, r_lo + XROWS)
            shard[blk * 64 : (blk + 1) * 64, g0 - r_lo : g1 - r_lo, 3:131] = x[
                img, :, g0:g1, :
            ]
        in_maps.append(
            dict(
                xs=shard.reshape(128, XROWS * PITCH).astype(bf),
                ow=ow, sel=sel, wt=wt, ob=ob, gb=gb,
            )
        )
    return in_maps


def kernel(x, offset_w, offset_b, dcn_w, gamma, beta):
    from concourse.bass_utils import run_bass_kernel_spmd

    if "nc" not in _CACHE:
        _CACHE["nc"] = _build_program()
    nc = _CACHE["nc"]

    in_maps = _shard_inputs(x, offset_w, offset_b, dcn_w, gamma, beta)
    res = run_bass_kernel_spmd(nc, in_maps, core_ids=list(range(NCORES)))
    out = np.zeros((B, C, H, W), np.float32)
    for core in range(NCORES):
        pair, q = core // 4, core % 4
        o = res.results[core]["out"].astype(np.float32).reshape(128, RPC, 128)
        for blk in range(2):
            out[2 * pair + blk, :, 32 * q : 32 * q + 32, :] = o[
                blk * 64 : (blk + 1) * 64
            ]
    return out
